# revision 38
# baseline (speedup 1.0000x reference)
"""Bass/Tile Trainium2 kernel for nn_CrossAttention (B=4, Nq=Nk=2048, D=1024, H=16).

v3 sharding: 8 cores; core c handles batch b=c//2 and head-group g=c%2
(heads 8g..8g+8) over ALL nq=2048 queries. Each core emits a PARTIAL output
(contraction over its 512 head-dims) in bf16; the host sums the pair and adds
the bias.

Ragged-sequence optimization: valid keys (attention_mask==1) are packed on the
host; pad rows get a -60 additive bias pre-exp on the last `nbias` key tiles.

v3 changes vs v2:
  - scores matmuls are row-tiled: the two heads of a pair run as CONCURRENT
    K=64 matmuls (tile_position (0,0) and (64,0)) instead of zero-padded K=128
    serial matmuls -> ~2x faster score phase on the PE.
  - qh kept as a single [128, nq] tile (no zero halves / memsets).
  - softmax reciprocal reads the AV PSUM rows directly (drops one DVE copy).
  - vh / out-proj PSUM evacuation copies routed to ScalarE (idle in those
    phases); kh/qh copies on the any-engine for scheduler balancing.
  - partial outputs written in bf16 (halves the tail DMA).
"""
import numpy as np
import ml_dtypes

import concourse.bass as bass
import concourse.mybir as mybir
import concourse.tile as tile
from concourse import bacc
from concourse.bass_utils import run_bass_kernel_spmd

F32 = mybir.dt.float32
BF16 = mybir.dt.bfloat16
NPBF16 = ml_dtypes.bfloat16

B, NQ_FULL, NK_FULL, D, H, DH = 4, 2048, 2048, 1024, 16, 64
SCALE = DH ** -0.5
MASK_NEG = -60.0
# DVE approximate-exp constants: bits_i16 = round(x*C1 + C2) bitcast to bf16
# approximates exp(x*SCALE) (Schraudolph exp2 trick, ~2% rms error). The
# rounding is done in pure fp32 by adding 2^23 (Knuth trick) so the integer
# lands in the mantissa — no float->int conversion, whose hardware semantics
# differ from CoreSim. The low halfword is then moved with a pure bit-copy.
LOG2E = 1.4426950408889634
EXP_C1 = SCALE * LOG2E * 128.0
EXP_C2 = (127.0 - 0.0573) * 128.0 + 8388608.0  # + 2^23
N_CORES = 8
HL = H // 2          # heads per core (8)
OW = HL * DH         # projection output width per core (512)


def _chunks(n, w=512):
    out, j = [], 0
    while j < n:
        out.append((j, min(w, n - j)))
        j += min(w, n - j)
    return out


def build_nc(nq, nk, nbias=2, d=D):
    """Per-core Bass program: nq query rows, nk packed key rows, 8 heads."""
    dh = DH
    assert d % 128 == 0 and nk % 128 == 0
    IC = d // 128          # contraction chunks for QKV projections
    NKT = nk // 128        # key tiles
    QC = max(1, nq // 512)
    QW = min(nq, 512)
    HP = HL // 2           # head pairs (4)
    OC = OW // 128         # avT dim-chunks (4) == out-proj contraction chunks
    nbias = min(nbias, NKT)

    nc = bacc.Bacc("TRN2", target_bir_lowering=False, debug=False)

    # activations arrive pre-transposed from the host: [d, n] layout
    xq = nc.declare_dram_parameter("xq", [d, nq], BF16, isOutput=False)
    xk = nc.declare_dram_parameter("xk", [d, nk], BF16, isOutput=False)
    xv = nc.declare_dram_parameter("xv", [d, nk], BF16, isOutput=False)
    wq = nc.declare_dram_parameter("wq", [d, OW], BF16, isOutput=False)  # Wq.T slice
    wk = nc.declare_dram_parameter("wk", [d, OW], BF16, isOutput=False)
    wv = nc.declare_dram_parameter("wv", [d, OW], BF16, isOutput=False)
    wp = nc.declare_dram_parameter("wp", [OW, d], BF16, isOutput=False)  # Wp.T rows
    maskb = nc.declare_dram_parameter("maskb", [128, NKT], F32, isOutput=False)
    # partial output stored TRANSPOSED [d, nq] so the out-proj can keep Wp
    # stationary across query chunks
    out = nc.declare_dram_parameter("out", [d, nq], BF16, isOutput=True)

    with tile.TileContext(nc) as tc:
        with (
            tc.tile_pool(name="wpool", bufs=1) as wpool,
            tc.tile_pool(name="const", bufs=1) as cpool,
            tc.tile_pool(name="acts", bufs=1) as apool,
            tc.tile_pool(name="xT", bufs=1) as xpool,
            tc.tile_pool(name="mm_ps", bufs=2, space="PSUM") as mmps,
        ):
            avT_s = apool.tile([128, OC, nq], BF16, tag="avT")
            vh_s = apool.tile([128, NKT, HL, 128], BF16, tag="vh")
            # all four head-pairs' kh kept resident: K projections run
            # upfront so their matmuls can fill early-pipeline bubbles
            kh_bufs = [apool.tile([128, nk], BF16, tag=f"kh{i}", name=f"kh{i}")
                       for i in range(HP)]
            # two alternating zero-padded qh tiles; the zero halves are
            # written once and persist across head-pairs (scores then run as
            # shared-lhsT K=128 matmuls: slot 0 = [qhA; 0], slot 1 = [0; qhB]).
            # Zeroed on the otherwise-idle GpSimd engine so the DVE is free
            # for the early PSUM-evacuation copies.
            qh_bufs = [apool.tile([128, 2, nq], BF16, tag=f"qh{i}", name=f"qh{i}")
                       for i in range(2)]
            for qz in qh_bufs:
                nc.gpsimd.memset(qz[0:64, 1, :], 0.0)
                nc.gpsimd.memset(qz[64:128, 0, :], 0.0)

            # ---- allocate ALL outer-pool tiles BEFORE opening the scoped
            # vpool: the stack allocator un-reserves anything allocated in an
            # outer pool during an inner pool's lifetime when that inner pool
            # is released, corrupting it later ----
            maskb_s = cpool.tile([128, NKT], F32, tag="maskb")
            wk_s = wpool.tile([128, IC, OW], BF16, tag="Wk", name="wk_s")
            wq_s = wpool.tile([128, IC, OW], BF16, tag="Wq", name="wq_s")
            wp_s = wpool.tile([128, OC, d], BF16, tag="Wp", name="wp_s")
            xkT = xpool.tile([128, IC, nk], BF16, tag="xkT", name="xkT")
            xqT = xpool.tile([128, IC, nq], BF16, tag="xqT", name="xqT")

            with (
                tc.tile_pool(name="vpool", bufs=1) as vpool,
                tc.tile_pool(name="warm_ps", bufs=1, space="PSUM") as wps,
            ):
                # PE warm-up: dummy matmuls keep the PE HAM-active while the
                # input DMAs land, so real work starts at the 2.4 GHz clock
                scr = vpool.tile([128, 512], BF16, tag="scr", name="scr")
                nc.vector.memset(scr[:, :], 0.0)
                wp0 = wps.tile([128, 512], F32, tag="w", name="warm")
                for i in range(18):
                    nc.tensor.matmul(wp0[:, :], scr[:, 0:128], scr[:, :],
                                     start=True, stop=True)
                # V-projection inputs get DMA priority (issued first)
                wv_s = vpool.tile([128, IC, OW], BF16, tag="Wv", name="wv_s")
                xvT = vpool.tile([128, IC, nk], BF16, tag="xvT", name="xvT")
                for ic in range(IC):
                    nc.sync.dma_start(out=wv_s[:, ic, :],
                                      in_=wv[ic * 128:(ic + 1) * 128, :])
                # xv sliced in column halves so the first V-proj tiles can
                # start before the whole tensor lands
                nkh = (NKT // 2) * 128
                for ic in range(IC):
                    nc.sync.dma_start(out=xvT[:, ic, 0:nkh],
                                      in_=xv[ic * 128:(ic + 1) * 128, 0:nkh])
                nc.sync.dma_start(out=maskb_s[:, :], in_=maskb[:, :])
                for ic in range(IC):
                    nc.sync.dma_start(out=xvT[:, ic, nkh:nk],
                                      in_=xv[ic * 128:(ic + 1) * 128, nkh:nk])
                for ic in range(IC):
                    nc.sync.dma_start(out=wk_s[:, ic, :],
                                      in_=wk[ic * 128:(ic + 1) * 128, :])
                    nc.sync.dma_start(out=xkT[:, ic, :],
                                      in_=xk[ic * 128:(ic + 1) * 128, :])
                for ic in range(IC):
                    nc.sync.dma_start(out=wq_s[:, ic, :],
                                      in_=wq[ic * 128:(ic + 1) * 128, :])
                    nc.sync.dma_start(out=xqT[:, ic, :],
                                      in_=xq[ic * 128:(ic + 1) * 128, :])


                # ---- V projection (8 heads, 512 outs): vh = [value | ones] ----
                for kt in range(NKT):
                    nc.gpsimd.memset(vh_s[:, kt, :, dh:], 1.0)
                    ps = mmps.tile([128, OW], F32, tag="ps", name=f"vps{kt}")
                    for ic in range(IC):
                        nc.tensor.matmul(
                            ps[:, :],
                            xvT[:, ic, kt * 128:(kt + 1) * 128],
                            wv_s[:, ic, :],
                            start=(ic == 0), stop=(ic == IC - 1),
                        )
                    nc.vector.tensor_copy(
                        vh_s[:, kt, :, 0:dh],
                        ps[:, :].rearrange("p (a b) -> p a b", b=dh),
                    )

            def proj_pairs(w_s, wc0, xT, copy_fn, n, tag):
                # chunk-PAIR groups with ic-outer loops: the stationary
                # weight tile is loaded once per (ic, group) and reused
                # for both chunks, keeping LDWEIGHTS off the critical path
                chunks = _chunks(n)
                for g0 in range(0, len(chunks), 2):
                    grp = chunks[g0:g0 + 2]
                    pss = [
                        mmps.tile([128, 512], F32, tag="ps",
                                  name=f"{tag}_{j0}")
                        for j0, _ in grp
                    ]
                    for ic in range(IC):
                        for (j0, jw), ps in zip(grp, pss):
                            nc.tensor.matmul(
                                ps[:, :jw],
                                w_s[:, ic, wc0:wc0 + 128],
                                xT[:, ic, j0:j0 + jw],
                                start=(ic == 0), stop=(ic == IC - 1),
                            )
                    for (j0, jw), ps in zip(grp, pss):
                        copy_fn(j0, jw, ps)

            # ---- ALL K projections upfront (their matmuls also serve as
            # pipeline filler for the first attention ramp) ----
            for hp in range(HP):
                kh_p = kh_bufs[hp]
                proj_pairs(
                    wk_s, hp * 128, xkT,
                    lambda j0, jw, ps, kh_p=kh_p: nc.vector.tensor_copy(
                        kh_p[:, j0:j0 + jw], ps[:, :jw]),
                    nk, f"kps{hp}")

            # ---- head-pair loop: Q projection + attention ----
            with (
                tc.tile_pool(name="epool", bufs=3) as epool,
                tc.tile_pool(name="sc_ps", bufs=2, space="PSUM") as scps,
                tc.tile_pool(name="av_ps", bufs=2, space="PSUM") as avps,
                tc.tile_pool(name="rpool", bufs=2) as rpool,
            ):
                for hp in range(HP):
                    if hp == 1:
                        # Wp load issued mid-attention: off the input-phase
                        # queues, but lands long before the out-projection
                        for ic in range(OC):
                            nc.sync.dma_start(
                                out=wp_s[:, ic, :],
                                in_=wp[ic * 128:(ic + 1) * 128, :])
                    kh_p = kh_bufs[hp]
                    qh = qh_bufs[hp % 2]

                    def qcopy(j0, jw, ps):
                        # ScalarE: idle during projection phases, keeps the
                        # DVE free so scores aren't gated on qh evacuation
                        nc.scalar.copy(qh[0:64, 0, j0:j0 + jw],
                                       ps[0:64, :jw])
                        nc.scalar.copy(qh[64:128, 1, j0:j0 + jw],
                                       ps[64:128, :jw])

                    proj_pairs(wq_s, hp * 128, xqT, qcopy, nq, f"qps{hp}")

                    for j in range(QC):
                        q0 = j * QW
                        es = [
                            epool.tile([128, NKT, QW], BF16, tag="e",
                                       name=f"e{hp}_{j}_{hf}")
                            for hf in range(2)
                        ]
                        for kp in range((NKT + 1) // 2):
                            kts = [kt for kt in (2 * kp, 2 * kp + 1) if kt < NKT]
                            pss = [
                                scps.tile([128, 2, 512], F32, tag="sc",
                                          name=f"sc{hp}_{j}_{kp}_{hf}")
                                for hf in range(2)
                            ]
                            for si, kt in enumerate(kts):
                                # both halves share the same stationary kh
                                # tile; the zero-padded qh keeps heads apart
                                for half, ps in enumerate(pss):
                                    nc.tensor.matmul(
                                        ps[:, si, :QW],
                                        kh_p[:, kt * 128:(kt + 1) * 128],
                                        qh[:, half, q0:q0 + QW],
                                        start=True, stop=True,
                                    )
                            for half, e in enumerate(es):
                                ps = pss[half]
                                if kts[-1] < NKT - nbias and len(kts) == 2:
                                    if kp == 2:
                                        # offload this pair's exp: DVE does the
                                        # affine+round in fp32, GpSimd (idle)
                                        # moves the low halfwords into the
                                        # bf16 E tile as a pure bit-copy
                                        ue = rpool.tile(
                                            [128, 2, QW], F32, tag="ue",
                                            name=f"ue{hp}_{j}_{half}")
                                        nc.vector.tensor_scalar(
                                            out=ue[:, :, :],
                                            in0=ps[:, :, :QW],
                                            scalar1=EXP_C1,
                                            scalar2=EXP_C2,
                                            op0=mybir.AluOpType.mult,
                                            op1=mybir.AluOpType.add,
                                        )
                                        u16 = ue[:, :, :].bitcast(
                                            mybir.dt.int16).rearrange(
                                            "p a (q two) -> p a q two", two=2)
                                        nc.gpsimd.tensor_copy(
                                            e[:, kts[0]:kts[0] + 2, :]
                                                .bitcast(mybir.dt.int16),
                                            u16[:, :, :, 0],
                                        )
                                    else:
                                        nc.scalar.activation(
                                            e[:, kts[0]:kts[0] + 2, :],
                                            ps[:, :, :QW],
                                            mybir.ActivationFunctionType.Exp,
                                            bias=0.0, scale=SCALE,
                                        )
                                else:
                                    for si, kt in enumerate(kts):
                                        if kt >= NKT - nbias:
                                            nc.scalar.activation(
                                                e[:, kt, :], ps[:, si, :QW],
                                                mybir.ActivationFunctionType.Exp,
                                                bias=maskb_s[:, kt:kt + 1],
                                                scale=SCALE,
                                            )
                                        else:
                                            nc.scalar.activation(
                                                e[:, kt, :], ps[:, si, :QW],
                                                mybir.ActivationFunctionType.Exp,
                                                bias=0.0, scale=SCALE,
                                            )
                        for half, e in enumerate(es):
                            hh = 2 * hp + half
                            av = avps.tile([128, 512], F32, tag="av",
                                           name=f"av{hp}_{j}_{half}")
                            for kt in range(NKT):
                                nc.tensor.matmul(
                                    av[:, :QW],
                                    vh_s[:, kt, hh, :],
                                    e[:, kt, :],
                                    start=(kt == 0), stop=(kt == NKT - 1),
                                )
                            # value rows 0:64; softmax denominators rows 64:128
                            dn = rpool.tile([128, QW], F32, tag="dn",
                                            name=f"dn_{hp}_{j}_{half}")
                            nc.vector.tensor_copy(dn[64:128, :], av[64:128, :QW])
                            dmv = rpool.tile([64, QW], F32, tag="dmv",
                                             name=f"dmv_{hp}_{j}_{half}")
                            nc.sync.dma_start(out=dmv[:, :], in_=dn[64:128, :])
                            rb = rpool.tile([64, QW], F32, tag="rb",
                                            name=f"rb_{hp}_{j}_{half}")
                            nc.vector.reciprocal_approx_fast(
                                out=rb[:, :], in_=dmv[:, :])
                            if half == 0:
                                nc.vector.tensor_mul(
                                    avT_s[0:dh, hp, q0:q0 + QW],
                                    av[0:dh, :QW], rb[:, :],
                                )
                            else:
                                avn = rpool.tile([dh, QW], BF16, tag="avn",
                                                 name=f"avn_{hp}_{j}")
                                nc.vector.tensor_mul(avn[:, :], av[0:dh, :QW],
                                                     rb[:, :])
                                nc.sync.dma_start(
                                    out=avT_s[64:128, hp, q0:q0 + QW],
                                    in_=avn[:, :],
                                )

            # ---- output projection, TRANSPOSED: out[d, nq] partial ----
            # Wp chunk is the stationary operand, reused over 4 query chunks
            with (
                tc.tile_pool(name="o_ps", bufs=4, space="PSUM") as ops,
                tc.tile_pool(name="obuf", bufs=4) as obuf,
            ):
                for oc in range(d // 128):
                    pss = [
                        ops.tile([128, QW], F32, tag="o", name=f"o{oc}_{j}")
                        for j in range(QC)
                    ]
                    for dc in range(OC):
                        for j, ps in enumerate(pss):
                            nc.tensor.matmul(
                                ps[:, :],
                                wp_s[:, dc, oc * 128:(oc + 1) * 128],
                                avT_s[:, dc, j * QW:(j + 1) * QW],
                                start=(dc == 0), stop=(dc == OC - 1),
                            )
                    for j, ps in enumerate(pss):
                        ot = obuf.tile([128, QW], BF16, tag="ot",
                                       name=f"ot{oc}_{j}")
                        if j % 2 == 0:
                            nc.scalar.copy(ot[:, :], ps[:, :])
                        else:
                            nc.vector.tensor_copy(ot[:, :], ps[:, :])
                        nc.sync.dma_start(
                            out=out[oc * 128:(oc + 1) * 128,
                                    j * QW:(j + 1) * QW],
                            in_=ot[:, :],
                        )

    nc.compile()
    return nc


def host_prep(q, k, v, attention_mask, Wq, Wk, Wv, Wp, bp):
    """Pack valid keys; slice weights by head-group; per-core input maps."""
    bsz, nk_full = attention_mask.shape
    idxs = [np.flatnonzero(attention_mask[b]) for b in range(bsz)]
    nv_min = min(len(ix) for ix in idxs)
    nk = max(128, -(-max(len(ix) for ix in idxs) // 128) * 128)
    nkt = nk // 128
    nbias = max(1, -(-(nk - nv_min) // 128))

    wqT = np.ascontiguousarray(Wq.T).astype(NPBF16)
    wkT = np.ascontiguousarray(Wk.T).astype(NPBF16)
    wvT = np.ascontiguousarray(Wv.T).astype(NPBF16)
    wpT = np.ascontiguousarray(Wp.T).astype(NPBF16)

    packed = []
    for b in range(bsz):
        ix = idxs[b]
        kp = np.zeros((nk, k.shape[2]), NPBF16)
        vp = np.zeros((nk, v.shape[2]), NPBF16)
        kp[:len(ix)] = k[b][ix].astype(NPBF16)
        vp[:len(ix)] = v[b][ix].astype(NPBF16)
        mb = np.full(nk, MASK_NEG, np.float32)
        mb[:len(ix)] = 0.0
        packed.append((np.ascontiguousarray(kp.T), np.ascontiguousarray(vp.T),
                       np.ascontiguousarray(mb.reshape(nkt, 128).T),
                       np.ascontiguousarray(q[b].astype(NPBF16).T)))

    in_maps = []
    for c in range(N_CORES):
        b, g = divmod(c, 2)
        kp, vp, mb, xqb = packed[b]
        o0 = g * OW
        in_maps.append({
            "xq": xqb, "xk": kp, "xv": vp,
            "wq": np.ascontiguousarray(wqT[:, o0:o0 + OW]),
            "wk": np.ascontiguousarray(wkT[:, o0:o0 + OW]),
            "wv": np.ascontiguousarray(wvT[:, o0:o0 + OW]),
            "wp": np.ascontiguousarray(wpT[o0:o0 + OW, :]),
            "maskb": mb,
        })
    return in_maps, nk, nbias


_NC_CACHE = {}


def get_nc(nq, nk, nbias=2):
    key = (nq, nk, nbias)
    if key not in _NC_CACHE:
        _NC_CACHE[key] = build_nc(nq, nk, nbias)
    return _NC_CACHE[key]


def combine(results, bp):
    out = np.empty((B, NQ_FULL, D), np.float32)
    for b in range(B):
        out[b] = (results[2 * b]["out"].astype(np.float32).T
                  + results[2 * b + 1]["out"].astype(np.float32).T + bp)
    return out


def kernel(q, k, v, attention_mask, Wq, Wk, Wv, Wp, bp):
    in_maps, nk, nbias = host_prep(q, k, v, attention_mask, Wq, Wk, Wv, Wp, bp)
    nc = get_nc(NQ_FULL, nk, nbias)
    res = run_bass_kernel_spmd(nc, in_maps, core_ids=list(range(N_CORES)))
    return combine(res.results, np.asarray(bp, np.float32))


# revision 42
# speedup vs baseline: 1.2380x; 1.2380x over previous
"""Bass/Tile Trainium2 kernel for nn_CrossAttention (B=4, Nq=Nk=2048, D=1024, H=16).

v3 sharding: 8 cores; core c handles batch b=c//2 and head-group g=c%2
(heads 8g..8g+8) over ALL nq=2048 queries. Each core emits a PARTIAL output
(contraction over its 512 head-dims) in bf16; the host sums the pair and adds
the bias.

Ragged-sequence optimization: valid keys (attention_mask==1) are packed on the
host; pad rows get a -60 additive bias pre-exp on the last `nbias` key tiles.

v3 changes vs v2:
  - scores matmuls are row-tiled: the two heads of a pair run as CONCURRENT
    K=64 matmuls (tile_position (0,0) and (64,0)) instead of zero-padded K=128
    serial matmuls -> ~2x faster score phase on the PE.
  - qh kept as a single [128, nq] tile (no zero halves / memsets).
  - softmax reciprocal reads the AV PSUM rows directly (drops one DVE copy).
  - vh / out-proj PSUM evacuation copies routed to ScalarE (idle in those
    phases); kh/qh copies on the any-engine for scheduler balancing.
  - partial outputs written in bf16 (halves the tail DMA).
"""
import numpy as np
import ml_dtypes

import concourse.bass as bass
import concourse.mybir as mybir
import concourse.tile as tile
from concourse import bacc
from concourse.bass_utils import run_bass_kernel_spmd

F32 = mybir.dt.float32
BF16 = mybir.dt.bfloat16
NPBF16 = ml_dtypes.bfloat16

B, NQ_FULL, NK_FULL, D, H, DH = 4, 2048, 2048, 1024, 16, 64
SCALE = DH ** -0.5
MASK_NEG = -60.0
# DVE approximate-exp constants: bits_i16 = round(x*C1 + C2) bitcast to bf16
# approximates exp(x*SCALE) (Schraudolph exp2 trick, ~2% rms error). The
# rounding is done in pure fp32 by adding 2^23 (Knuth trick) so the integer
# lands in the mantissa — no float->int conversion, whose hardware semantics
# differ from CoreSim. The low halfword is then moved with a pure bit-copy.
LOG2E = 1.4426950408889634
EXP_C1 = SCALE * LOG2E * 128.0
EXP_C2 = (127.0 - 0.0573) * 128.0 + 8388608.0  # + 2^23
N_CORES = 8
HL = H // 2          # heads per core (8)
OW = HL * DH         # projection output width per core (512)


def _chunks(n, w=512):
    out, j = [], 0
    while j < n:
        out.append((j, min(w, n - j)))
        j += min(w, n - j)
    return out


def build_nc(nq, nk, nbias=2, d=D):
    """Per-core Bass program: nq query rows, nk packed key rows, 8 heads."""
    dh = DH
    assert d % 128 == 0 and nk % 128 == 0
    IC = d // 128          # contraction chunks for QKV projections
    NKT = nk // 128        # key tiles
    QC = max(1, nq // 512)
    QW = min(nq, 512)
    HP = HL // 2           # head pairs (4)
    OC = OW // 128         # avT dim-chunks (4) == out-proj contraction chunks
    nbias = min(nbias, NKT)

    nc = bacc.Bacc("TRN2", target_bir_lowering=False, debug=False)

    # activations arrive pre-transposed from the host: [d, n] layout
    xq = nc.declare_dram_parameter("xq", [d, nq], BF16, isOutput=False)
    xk = nc.declare_dram_parameter("xk", [d, nk], BF16, isOutput=False)
    xv = nc.declare_dram_parameter("xv", [d, nk], BF16, isOutput=False)
    wq = nc.declare_dram_parameter("wq", [d, OW], BF16, isOutput=False)  # Wq.T slice
    wk = nc.declare_dram_parameter("wk", [d, OW], BF16, isOutput=False)
    wv = nc.declare_dram_parameter("wv", [d, OW], BF16, isOutput=False)
    wp = nc.declare_dram_parameter("wp", [OW, d], BF16, isOutput=False)  # Wp.T rows
    maskb = nc.declare_dram_parameter("maskb", [128, NKT], F32, isOutput=False)
    # partial output stored TRANSPOSED [d, nq] so the out-proj can keep Wp
    # stationary across query chunks
    out = nc.declare_dram_parameter("out", [d, nq], BF16, isOutput=True)

    with tile.TileContext(nc) as tc:
        with (
            tc.tile_pool(name="wpool", bufs=1) as wpool,
            tc.tile_pool(name="const", bufs=1) as cpool,
            tc.tile_pool(name="acts", bufs=1) as apool,
            tc.tile_pool(name="xT", bufs=1) as xpool,
            tc.tile_pool(name="mm_ps", bufs=2, space="PSUM") as mmps,
        ):
            avT_s = apool.tile([128, OC, nq], BF16, tag="avT")
            vh_s = apool.tile([128, NKT, HL, 128], BF16, tag="vh")
            # all four head-pairs' kh kept resident: K projections run
            # upfront so their matmuls can fill early-pipeline bubbles
            kh_bufs = [apool.tile([128, nk], BF16, tag=f"kh{i}", name=f"kh{i}")
                       for i in range(HP)]
            # two alternating zero-padded qh tiles; the zero halves are
            # written once and persist across head-pairs (scores then run as
            # shared-lhsT K=128 matmuls: slot 0 = [qhA; 0], slot 1 = [0; qhB]).
            # Zeroed on the otherwise-idle GpSimd engine so the DVE is free
            # for the early PSUM-evacuation copies.
            qh_bufs = [apool.tile([128, 2, nq], BF16, tag=f"qh{i}", name=f"qh{i}")
                       for i in range(2)]
            for qz in qh_bufs:
                nc.gpsimd.memset(qz[0:64, 1, :], 0.0)
                nc.gpsimd.memset(qz[64:128, 0, :], 0.0)

            # ---- allocate ALL outer-pool tiles BEFORE opening the scoped
            # vpool: the stack allocator un-reserves anything allocated in an
            # outer pool during an inner pool's lifetime when that inner pool
            # is released, corrupting it later ----
            maskb_s = cpool.tile([128, NKT], F32, tag="maskb")
            wk_s = wpool.tile([128, IC, OW], BF16, tag="Wk", name="wk_s")
            wq_s = wpool.tile([128, IC, OW], BF16, tag="Wq", name="wq_s")
            wp_s = wpool.tile([128, OC, d], BF16, tag="Wp", name="wp_s")
            xkT = xpool.tile([128, IC, nk], BF16, tag="xkT", name="xkT")
            xqT = xpool.tile([128, IC, nq], BF16, tag="xqT", name="xqT")

            with (
                tc.tile_pool(name="vpool", bufs=1) as vpool,
                tc.tile_pool(name="warm_ps", bufs=1, space="PSUM") as wps,
            ):
                # PE warm-up: dummy matmuls keep the PE HAM-active while the
                # input DMAs land, so real work starts at the 2.4 GHz clock
                scr = vpool.tile([128, 512], BF16, tag="scr", name="scr")
                nc.vector.memset(scr[:, :], 0.0)
                wp0 = wps.tile([128, 512], F32, tag="w", name="warm")
                for i in range(18):
                    nc.tensor.matmul(wp0[:, :], scr[:, 0:128], scr[:, :],
                                     start=True, stop=True)
                # V-projection inputs get DMA priority (issued first)
                wv_s = vpool.tile([128, IC, OW], BF16, tag="Wv", name="wv_s")
                xvT = vpool.tile([128, IC, nk], BF16, tag="xvT", name="xvT")
                for ic in range(IC):
                    nc.sync.dma_start(out=wv_s[:, ic, :],
                                      in_=wv[ic * 128:(ic + 1) * 128, :])
                # xv sliced in column halves so the first V-proj tiles can
                # start before the whole tensor lands
                nkh = (NKT // 2) * 128
                for ic in range(IC):
                    nc.sync.dma_start(out=xvT[:, ic, 0:nkh],
                                      in_=xv[ic * 128:(ic + 1) * 128, 0:nkh])
                nc.sync.dma_start(out=maskb_s[:, :], in_=maskb[:, :])
                for ic in range(IC):
                    nc.sync.dma_start(out=xvT[:, ic, nkh:nk],
                                      in_=xv[ic * 128:(ic + 1) * 128, nkh:nk])
                for ic in range(IC):
                    nc.sync.dma_start(out=wk_s[:, ic, :],
                                      in_=wk[ic * 128:(ic + 1) * 128, :])
                    nc.sync.dma_start(out=xkT[:, ic, :],
                                      in_=xk[ic * 128:(ic + 1) * 128, :])
                for ic in range(IC):
                    nc.sync.dma_start(out=wq_s[:, ic, :],
                                      in_=wq[ic * 128:(ic + 1) * 128, :])
                    nc.sync.dma_start(out=xqT[:, ic, :],
                                      in_=xq[ic * 128:(ic + 1) * 128, :])


                # ---- V projection (8 heads, 512 outs): vh = [value | ones] ----
                for kt in range(NKT):
                    nc.gpsimd.memset(vh_s[:, kt, :, dh:], 1.0)
                    ps = mmps.tile([128, OW], F32, tag="ps", name=f"vps{kt}")
                    for ic in range(IC):
                        nc.tensor.matmul(
                            ps[:, :],
                            xvT[:, ic, kt * 128:(kt + 1) * 128],
                            wv_s[:, ic, :],
                            start=(ic == 0), stop=(ic == IC - 1),
                        )
                    nc.vector.tensor_copy(
                        vh_s[:, kt, :, 0:dh],
                        ps[:, :].rearrange("p (a b) -> p a b", b=dh),
                    )

            def proj_pairs(w_s, wc0, xT, copy_fn, n, tag):
                # chunk-PAIR groups with ic-outer loops: the stationary
                # weight tile is loaded once per (ic, group) and reused
                # for both chunks, keeping LDWEIGHTS off the critical path
                chunks = _chunks(n)
                for g0 in range(0, len(chunks), 2):
                    grp = chunks[g0:g0 + 2]
                    pss = [
                        mmps.tile([128, 512], F32, tag="ps",
                                  name=f"{tag}_{j0}")
                        for j0, _ in grp
                    ]
                    for ic in range(IC):
                        for (j0, jw), ps in zip(grp, pss):
                            nc.tensor.matmul(
                                ps[:, :jw],
                                w_s[:, ic, wc0:wc0 + 128],
                                xT[:, ic, j0:j0 + jw],
                                start=(ic == 0), stop=(ic == IC - 1),
                            )
                    for (j0, jw), ps in zip(grp, pss):
                        copy_fn(j0, jw, ps)

            # ---- ALL K projections upfront (their matmuls also serve as
            # pipeline filler for the first attention ramp) ----
            for hp in range(HP):
                kh_p = kh_bufs[hp]
                proj_pairs(
                    wk_s, hp * 128, xkT,
                    lambda j0, jw, ps, kh_p=kh_p: nc.vector.tensor_copy(
                        kh_p[:, j0:j0 + jw], ps[:, :jw]),
                    nk, f"kps{hp}")

            # ---- head-pair loop: Q projection + attention ----
            with (
                tc.tile_pool(name="epool", bufs=3) as epool,
                tc.tile_pool(name="sc_ps", bufs=2, space="PSUM") as scps,
                tc.tile_pool(name="av_ps", bufs=2, space="PSUM") as avps,
                tc.tile_pool(name="rpool", bufs=2) as rpool,
            ):
                for hp in range(HP):
                    if hp == 1:
                        # Wp load issued mid-attention: off the input-phase
                        # queues, but lands long before the out-projection
                        for ic in range(OC):
                            nc.sync.dma_start(
                                out=wp_s[:, ic, :],
                                in_=wp[ic * 128:(ic + 1) * 128, :])
                    kh_p = kh_bufs[hp]
                    qh = qh_bufs[hp % 2]

                    def qcopy(j0, jw, ps):
                        # ScalarE: idle during projection phases, keeps the
                        # DVE free so scores aren't gated on qh evacuation
                        nc.scalar.copy(qh[0:64, 0, j0:j0 + jw],
                                       ps[0:64, :jw])
                        nc.scalar.copy(qh[64:128, 1, j0:j0 + jw],
                                       ps[64:128, :jw])

                    proj_pairs(wq_s, hp * 128, xqT, qcopy, nq, f"qps{hp}")

                    for j in range(QC):
                        q0 = j * QW
                        es = [
                            epool.tile([128, NKT, QW], BF16, tag="e",
                                       name=f"e{hp}_{j}_{hf}")
                            for hf in range(2)
                        ]
                        ues = [None, None]
                        for kp in range((NKT + 1) // 2):
                            kts = [kt for kt in (2 * kp, 2 * kp + 1) if kt < NKT]
                            pss = [
                                scps.tile([128, 2, 512], F32, tag="sc",
                                          name=f"sc{hp}_{j}_{kp}_{hf}")
                                for hf in range(2)
                            ]
                            for si, kt in enumerate(kts):
                                # both halves share the same stationary kh
                                # tile; the zero-padded qh keeps heads apart
                                for half, ps in enumerate(pss):
                                    nc.tensor.matmul(
                                        ps[:, si, :QW],
                                        kh_p[:, kt * 128:(kt + 1) * 128],
                                        qh[:, half, q0:q0 + QW],
                                        start=True, stop=True,
                                    )
                            for half, e in enumerate(es):
                                ps = pss[half]
                                if kts[-1] < NKT - nbias and len(kts) == 2:
                                    if kp == 2:
                                        # exp offloaded to the DVE: affine +
                                        # Knuth 2^23 rounding in pure fp32;
                                        # the AV matmul later reads the low
                                        # halfwords through a strided bf16
                                        # view -- no extraction copy at all
                                        ue = rpool.tile(
                                            [128, 2, QW], F32, tag="ue",
                                            name=f"ue{hp}_{j}_{half}")
                                        nc.vector.tensor_scalar(
                                            out=ue[:, :, :],
                                            in0=ps[:, :, :QW],
                                            scalar1=EXP_C1,
                                            scalar2=EXP_C2,
                                            op0=mybir.AluOpType.mult,
                                            op1=mybir.AluOpType.add,
                                        )
                                        ues[half] = ue
                                    else:
                                        nc.scalar.activation(
                                            e[:, kts[0]:kts[0] + 2, :],
                                            ps[:, :, :QW],
                                            mybir.ActivationFunctionType.Exp,
                                            bias=0.0, scale=SCALE,
                                        )
                                else:
                                    for si, kt in enumerate(kts):
                                        if kt >= NKT - nbias:
                                            nc.scalar.activation(
                                                e[:, kt, :], ps[:, si, :QW],
                                                mybir.ActivationFunctionType.Exp,
                                                bias=maskb_s[:, kt:kt + 1],
                                                scale=SCALE,
                                            )
                                        else:
                                            nc.scalar.activation(
                                                e[:, kt, :], ps[:, si, :QW],
                                                mybir.ActivationFunctionType.Exp,
                                                bias=0.0, scale=SCALE,
                                            )
                        for half, e in enumerate(es):
                            hh = 2 * hp + half
                            av = avps.tile([128, 512], F32, tag="av",
                                           name=f"av{hp}_{j}_{half}")
                            for kt in range(NKT):
                                if ues[half] is not None and kt in (4, 5):
                                    # strided bf16 view of the fp32 ue tile:
                                    # low halfword of each word = approx exp
                                    rhs = (ues[half][:, kt - 4, :]
                                           .bitcast(mybir.dt.int16)
                                           .rearrange("p (q two) -> p q two",
                                                      two=2)[:, :, 0]
                                           .bitcast(BF16))
                                else:
                                    rhs = e[:, kt, :]
                                nc.tensor.matmul(
                                    av[:, :QW],
                                    vh_s[:, kt, hh, :],
                                    rhs,
                                    start=(kt == 0), stop=(kt == NKT - 1),
                                )
                            # value rows 0:64; softmax denominators rows 64:128
                            dn = rpool.tile([128, QW], F32, tag="dn",
                                            name=f"dn_{hp}_{j}_{half}")
                            nc.vector.tensor_copy(dn[64:128, :], av[64:128, :QW])
                            dmv = rpool.tile([64, QW], F32, tag="dmv",
                                             name=f"dmv_{hp}_{j}_{half}")
                            nc.sync.dma_start(out=dmv[:, :], in_=dn[64:128, :])
                            rb = rpool.tile([64, QW], F32, tag="rb",
                                            name=f"rb_{hp}_{j}_{half}")
                            nc.vector.reciprocal_approx_fast(
                                out=rb[:, :], in_=dmv[:, :])
                            if half == 0:
                                nc.vector.tensor_mul(
                                    avT_s[0:dh, hp, q0:q0 + QW],
                                    av[0:dh, :QW], rb[:, :],
                                )
                            else:
                                avn = rpool.tile([dh, QW], BF16, tag="avn",
                                                 name=f"avn_{hp}_{j}")
                                nc.vector.tensor_mul(avn[:, :], av[0:dh, :QW],
                                                     rb[:, :])
                                nc.sync.dma_start(
                                    out=avT_s[64:128, hp, q0:q0 + QW],
                                    in_=avn[:, :],
                                )

            # ---- output projection, TRANSPOSED: out[d, nq] partial ----
            # Wp chunk is the stationary operand, reused over 4 query chunks
            with (
                tc.tile_pool(name="o_ps", bufs=4, space="PSUM") as ops,
                tc.tile_pool(name="obuf", bufs=4) as obuf,
            ):
                for oc in range(d // 128):
                    pss = [
                        ops.tile([128, QW], F32, tag="o", name=f"o{oc}_{j}")
                        for j in range(QC)
                    ]
                    for dc in range(OC):
                        for j, ps in enumerate(pss):
                            nc.tensor.matmul(
                                ps[:, :],
                                wp_s[:, dc, oc * 128:(oc + 1) * 128],
                                avT_s[:, dc, j * QW:(j + 1) * QW],
                                start=(dc == 0), stop=(dc == OC - 1),
                            )
                    for j, ps in enumerate(pss):
                        ot = obuf.tile([128, QW], BF16, tag="ot",
                                       name=f"ot{oc}_{j}")
                        if j % 2 == 0:
                            nc.scalar.copy(ot[:, :], ps[:, :])
                        else:
                            nc.vector.tensor_copy(ot[:, :], ps[:, :])
                        nc.sync.dma_start(
                            out=out[oc * 128:(oc + 1) * 128,
                                    j * QW:(j + 1) * QW],
                            in_=ot[:, :],
                        )

    nc.compile()
    return nc


def host_prep(q, k, v, attention_mask, Wq, Wk, Wv, Wp, bp):
    """Pack valid keys; slice weights by head-group; per-core input maps."""
    bsz, nk_full = attention_mask.shape
    idxs = [np.flatnonzero(attention_mask[b]) for b in range(bsz)]
    nv_min = min(len(ix) for ix in idxs)
    nk = max(128, -(-max(len(ix) for ix in idxs) // 128) * 128)
    nkt = nk // 128
    nbias = max(1, -(-(nk - nv_min) // 128))

    wqT = np.ascontiguousarray(Wq.T).astype(NPBF16)
    wkT = np.ascontiguousarray(Wk.T).astype(NPBF16)
    wvT = np.ascontiguousarray(Wv.T).astype(NPBF16)
    wpT = np.ascontiguousarray(Wp.T).astype(NPBF16)

    packed = []
    for b in range(bsz):
        ix = idxs[b]
        kp = np.zeros((nk, k.shape[2]), NPBF16)
        vp = np.zeros((nk, v.shape[2]), NPBF16)
        kp[:len(ix)] = k[b][ix].astype(NPBF16)
        vp[:len(ix)] = v[b][ix].astype(NPBF16)
        mb = np.full(nk, MASK_NEG, np.float32)
        mb[:len(ix)] = 0.0
        packed.append((np.ascontiguousarray(kp.T), np.ascontiguousarray(vp.T),
                       np.ascontiguousarray(mb.reshape(nkt, 128).T),
                       np.ascontiguousarray(q[b].astype(NPBF16).T)))

    in_maps = []
    for c in range(N_CORES):
        b, g = divmod(c, 2)
        kp, vp, mb, xqb = packed[b]
        o0 = g * OW
        in_maps.append({
            "xq": xqb, "xk": kp, "xv": vp,
            "wq": np.ascontiguousarray(wqT[:, o0:o0 + OW]),
            "wk": np.ascontiguousarray(wkT[:, o0:o0 + OW]),
            "wv": np.ascontiguousarray(wvT[:, o0:o0 + OW]),
            "wp": np.ascontiguousarray(wpT[o0:o0 + OW, :]),
            "maskb": mb,
        })
    return in_maps, nk, nbias


_NC_CACHE = {}


def get_nc(nq, nk, nbias=2):
    key = (nq, nk, nbias)
    if key not in _NC_CACHE:
        _NC_CACHE[key] = build_nc(nq, nk, nbias)
    return _NC_CACHE[key]


def combine(results, bp):
    out = np.empty((B, NQ_FULL, D), np.float32)
    for b in range(B):
        out[b] = (results[2 * b]["out"].astype(np.float32).T
                  + results[2 * b + 1]["out"].astype(np.float32).T + bp)
    return out


def kernel(q, k, v, attention_mask, Wq, Wk, Wv, Wp, bp):
    in_maps, nk, nbias = host_prep(q, k, v, attention_mask, Wq, Wk, Wv, Wp, bp)
    nc = get_nc(NQ_FULL, nk, nbias)
    res = run_bass_kernel_spmd(nc, in_maps, core_ids=list(range(N_CORES)))
    return combine(res.results, np.asarray(bp, np.float32))
